# revision 5
# baseline (speedup 1.0000x reference)
"""Trainium2 Bass kernel for nn_Encoder_40535901340423 (binary-tree GRU encoder).

v2: same sharding/host packing as baseline (batch/8, feature-major on chip),
rewritten device schedule:
  - PE: weight-major over chunk PAIRS (one LDWEIGHTS serves 2 matmuls),
    512-col chunks, 4 PSUM banks per chunk, 2 chunks in flight.
  - ACT: one sigmoid per chunk over [r|z'] (1024 cols, PSUM src),
    one tanh per quad (<=2048 cols, SBUF).
  - DVE only for elementwise (no gpsimd compute - Pool ops poison DVE
    SBUF ports): all-bf16-SBUF tensor_tensor (2x mode) / tensor_scalar
    (4x) where possible; PSUM-source ops kept to t/u per chunk.
  - combine: s=h1+h2, m=0.5s, d=c-m, e=z'*d, h=m+e (all TT/TS).
"""

import math
import sys

import numpy as np
import ml_dtypes

if "/opt/trn_rl_repo" not in sys.path:
    sys.path.insert(0, "/opt/trn_rl_repo")

import concourse.bass as bass
from concourse import bacc
import concourse.mybir as mybir
import concourse.tile as tile

N_NODES, B, V, H = 2047, 128, 32, 128
NCORES = 8
BLOC = B // NCORES  # 16
CHUNK = 512
QUAD = 4  # chunks per combine quad
F32 = mybir.dt.float32
BF16 = mybir.dt.bfloat16
NPBF16 = ml_dtypes.bfloat16
AL = mybir.AluOpType
AF = mybir.ActivationFunctionType


def _level_meta():
    meta = []
    off = 0
    for d in range(10, -1, -1):
        n = 1 << d
        L = n * BLOC
        if L >= 2 * CHUNK:
            bs = int(math.ceil(L / 2 / CHUNK)) * CHUNK
        else:
            bs = L
        nblk = int(math.ceil(L / bs))
        meta.append(dict(d=d, n=n, L=L, bs=bs, nblk=nblk, xoff=off))
        off += bs
    return meta, off


LEVELS, XTOT = _level_meta()


def build_nc(apply_mask: bool):
    nc = bacc.Bacc()
    xp = nc.declare_dram_parameter("xp", [128, XTOT], BF16, isOutput=False)
    wx = nc.declare_dram_parameter("wx", [128, 6 * H], BF16, isOutput=False)
    wh = nc.declare_dram_parameter("wh", [128, 6 * H], BF16, isOutput=False)
    whd = nc.declare_dram_parameter("whd", [128, 2 * H], BF16, isOutput=False)
    bia = nc.declare_dram_parameter("bias", [128, 3], F32, isOutput=False)
    mrow = None
    if apply_mask:
        mrow = nc.declare_dram_parameter(
            "mrow", [1, N_NODES * BLOC], F32, isOutput=False)
    out = nc.declare_dram_parameter("out", [2, 128, BLOC], F32, isOutput=True)

    with tile.TileContext(nc) as tc:
        _emit(tc, nc, xp, wx, wh, whd, bia, mrow, out, apply_mask)
    if not nc.is_finalized():
        nc.finalize()
    return nc


def _emit(tc, nc, xp, wx, wh, whd, bia, mrow, out, apply_mask):
    import contextlib

    with contextlib.ExitStack() as ctx:
        singles = ctx.enter_context(tc.tile_pool(name="singles", bufs=1))
        hbufs = ctx.enter_context(tc.tile_pool(name="hbufs", bufs=1))
        ps_rz = ctx.enter_context(tc.tile_pool(name="ps_rz", bufs=2, space="PSUM"))
        ps_hn = ctx.enter_context(tc.tile_pool(name="ps_hn", bufs=2, space="PSUM"))
        ps_xn = ctx.enter_context(tc.tile_pool(name="ps_xn", bufs=2, space="PSUM"))
        sb_rz = ctx.enter_context(tc.tile_pool(name="sb_rz", bufs=2))
        sb_t = ctx.enter_context(tc.tile_pool(name="sb_t", bufs=2))
        sb_u = ctx.enter_context(tc.tile_pool(name="sb_u", bufs=2))
        sb_c = ctx.enter_context(tc.tile_pool(name="sb_c", bufs=2))
        sb_s = ctx.enter_context(tc.tile_pool(name="sb_s", bufs=2))
        sb_m = ctx.enter_context(tc.tile_pool(name="sb_m", bufs=2))

        # --- resident tensors ---
        wx_t = singles.tile([128, 6 * H], BF16, tag="wx")
        nc.sync.dma_start(out=wx_t[:, :], in_=wx[:, :])
        wh_t = singles.tile([128, 6 * H], BF16, tag="wh")
        nc.sync.dma_start(out=wh_t[:, :], in_=wh[:, :])
        whd_t = singles.tile([128, 2 * H], BF16, tag="whd")
        nc.sync.dma_start(out=whd_t[:, :], in_=whd[:, :])
        bia_t = singles.tile([128, 3], F32, tag="bias")
        nc.sync.dma_start(out=bia_t[:, :], in_=bia[:, :])
        x_res = singles.tile([128, XTOT], BF16, tag="x_res")
        for lv in LEVELS:
            half = lv["bs"] // 8 if lv["d"] == 10 else lv["bs"]
            for p0 in range(lv["xoff"], lv["xoff"] + lv["bs"], max(half, 16)):
                p1 = min(p0 + max(half, 16), lv["xoff"] + lv["bs"])
                nc.sync.dma_start(out=x_res[:, p0:p1], in_=xp[:, p0:p1])
        whn_b = bia_t[:, 0:1]
        mu_b = bia_t[:, 1:2]
        lv_b = bia_t[:, 2:3]

        ping = hbufs.tile([128, 16384], BF16, tag="ping")
        pong = hbufs.tile([128, 8192], BF16, tag="pong")

        def hbuf(d):
            L = (1 << d) * BLOC
            return (ping if (10 - d) % 2 == 0 else pong)[:, :L]

        WXg = [[wx_t[:, (3 * bI + g) * H:(3 * bI + g + 1) * H]
                for g in range(3)] for bI in range(2)]
        Ug = [wh_t[:, (2 * g) * H:(2 * g + 1) * H] for g in range(3)]
        Vg = [wh_t[:, (2 * g + 1) * H:(2 * g + 2) * H] for g in range(3)]

        def mask_tile(d, c0, sz, pool, nm):
            n = 1 << d
            start = (n - 1) * BLOC
            m_t = pool.tile([128, QUAD * CHUNK], F32, tag="m", name=nm)
            src = mrow[0:1, start + c0: start + c0 + sz]
            bsrc = bass.AP(tensor=src.tensor, offset=src.offset,
                           ap=[[0, 128]] + list(src.ap[1:]))
            nc.sync.dma_start(out=m_t[:, :sz], in_=bsrc)
            return m_t

        class LevelCtx:
            def __init__(self, lv):
                self.lv = lv
                self.d, self.L = lv["d"], lv["L"]
                self.bs, self.xoff = lv["bs"], lv["xoff"]
                self.leaf = self.d == 10
                self.h_out = hbuf(self.d)
                self.hv = None
                if not self.leaf:
                    self.hv = hbuf(self.d + 1).rearrange(
                        "p (n two b) -> p n two b", two=2, b=BLOC)
                self.nchunks = int(math.ceil(self.L / CHUNK))
                self.chunks = [(i * CHUNK, min((i + 1) * CHUNK, self.L))
                               for i in range(self.nchunks)]
                if 64 <= self.L <= CHUNK:
                    half = -(-self.L // 2 // BLOC) * BLOC
                    self.chunks = [(0, half), (half, self.L)]
                    self.nchunks = 2
                self.npairs = int(math.ceil(self.nchunks / 2))
                self.nquads = int(math.ceil(self.nchunks / QUAD))
                # per-quad state: rz/t/u tiles
                self.q_rz = [None] * self.nquads
                self.q_t = [None] * self.nquads
                self.q_u = [None] * self.nquads

            def xin(self, g, c0, c1):
                """(lhsT, rhs) for the x matmul of gate g over level cols
                [c0,c1) (must lie within one block)."""
                bI = c0 // self.bs
                o = self.xoff + c0 - bI * self.bs
                return (WXg[bI][g], x_res[:, o:o + (c1 - c0)])

            def quad_tiles(self, qi):
                if self.q_rz[qi] is None:
                    self.q_rz[qi] = sb_rz.tile(
                        [128, 2 * QUAD * CHUNK], BF16, tag="rzq",
                        name=f"rzq{self.d}_{qi}")
                    self.q_t[qi] = sb_t.tile(
                        [128, QUAD * CHUNK], BF16, tag="tq",
                        name=f"tq{self.d}_{qi}")
                    self.q_u[qi] = sb_u.tile(
                        [128, QUAD * CHUNK], BF16, tag="uq",
                        name=f"uq{self.d}_{qi}")
                return self.q_rz[qi], self.q_t[qi], self.q_u[qi]

            def emit_pair(self, pi):
                """Emit PE + sigmoid + t/u for chunks 2pi, 2pi+1."""
                cidx = [c for c in (2 * pi, 2 * pi + 1) if c < self.nchunks]
                chs = [self.chunks[c] for c in cidx]
                leaf = self.leaf
                rzs = []
                hns = []
                xns = []
                kids = []
                for (c0, c1) in chs:
                    sz = c1 - c0
                    rzs.append(ps_rz.tile([128, 2 * CHUNK], F32, tag="rz",
                                          name=f"rz{self.d}_{c0}"))
                    xns.append(ps_xn.tile([128, CHUNK], F32, tag="xn",
                                          name=f"xn{self.d}_{c0}"))
                    if not leaf:
                        hns.append(ps_hn.tile([128, CHUNK], F32, tag="hn",
                                              name=f"hn{self.d}_{c0}"))
                        n0, n1 = c0 // BLOC, c1 // BLOC
                        kids.append((self.hv[:, n0:n1, 0, :],
                                     self.hv[:, n0:n1, 1, :]))
                # r then z' gates: x (start) -> U -> V (stop), weight-major
                for g in range(2):
                    for k, (c0, c1) in enumerate(chs):
                        sz = c1 - c0
                        lhsT, rhs = self.xin(g, c0, c1)
                        nc.tensor.matmul(rzs[k][:, g * sz:(g + 1) * sz],
                                         lhsT, rhs, start=True, stop=leaf)
                    if not leaf:
                        for k, (c0, c1) in enumerate(chs):
                            sz = c1 - c0
                            nc.tensor.matmul(rzs[k][:, g * sz:(g + 1) * sz],
                                             Ug[g], kids[k][0],
                                             start=False, stop=False)
                        for k, (c0, c1) in enumerate(chs):
                            sz = c1 - c0
                            nc.tensor.matmul(rzs[k][:, g * sz:(g + 1) * sz],
                                             Vg[g], kids[k][1],
                                             start=False, stop=True)
                # sigmoid per chunk: [r|z'] -> rz quad tile (r half / z half)
                for k, c in enumerate(cidx):
                    c0, c1 = self.chunks[c]
                    sz = c1 - c0
                    qi, qslot = c // QUAD, c % QUAD
                    rzq, _, _ = self.quad_tiles(qi)
                    qoff = c0 - self.chunks[qi * QUAD][0]
                    rz_out = rzq.rearrange("p (b c) -> p b c", b=2)[
                        :, :, qoff:qoff + sz]
                    rz_in = rzs[k][:, :2 * sz].rearrange(
                        "p (b c) -> p b c", b=2)
                    nc.scalar.activation(rz_out, rz_in, AF.Sigmoid)
                # xn gate
                for k, (c0, c1) in enumerate(chs):
                    sz = c1 - c0
                    lhsT, rhs = self.xin(2, c0, c1)
                    nc.tensor.matmul(xns[k][:, :sz], lhsT, rhs,
                                     start=True, stop=True)
                if not leaf:
                    for k, (c0, c1) in enumerate(chs):
                        sz = c1 - c0
                        nc.tensor.matmul(hns[k][:, :sz], Ug[2], kids[k][0],
                                         start=True, stop=False)
                    for k, (c0, c1) in enumerate(chs):
                        sz = c1 - c0
                        nc.tensor.matmul(hns[k][:, :sz], Vg[2], kids[k][1],
                                         start=False, stop=True)
                # t / u per chunk on DVE
                for k, c in enumerate(cidx):
                    c0, c1 = self.chunks[c]
                    sz = c1 - c0
                    qi = c // QUAD
                    rzq, tq, uq = self.quad_tiles(qi)
                    qoff = c0 - self.chunks[qi * QUAD][0]
                    r_sl = rzq[:, qoff:qoff + sz]
                    if leaf:
                        # u = r*whn_b + xn
                        nc.vector.scalar_tensor_tensor(
                            uq[:, qoff:qoff + sz], r_sl, whn_b,
                            xns[k][:, :sz], AL.mult, AL.add)
                    else:
                        # t = (hn + whn_b) * r ; u = t + xn
                        nc.vector.scalar_tensor_tensor(
                            tq[:, qoff:qoff + sz], hns[k][:, :sz], whn_b,
                            r_sl, AL.add, AL.mult)
                        nc.vector.tensor_tensor(
                            uq[:, qoff:qoff + sz], tq[:, qoff:qoff + sz],
                            xns[k][:, :sz], AL.add)
                last = cidx[-1]
                if last % QUAD == QUAD - 1 or last == self.nchunks - 1:
                    return (last // QUAD, last)
                return None

            def emit_quad(self, qi, ci_last):
                qc0 = self.chunks[qi * QUAD][0]
                qc1 = self.chunks[ci_last][1]
                qcols = qc1 - qc0
                rzq, tq, uq = self.quad_tiles(qi)
                z_q = rzq[:, QUAD * CHUNK:QUAD * CHUNK + qcols]
                c_q = sb_c.tile([128, QUAD * CHUNK], BF16, tag="cq",
                                name=f"cq{self.d}_{qi}")
                nc.scalar.activation(c_q[:, :qcols], uq[:, :qcols], AF.Tanh)
                hsl = self.h_out[:, qc0:qc1]
                m_t = None
                if apply_mask:
                    m_t = mask_tile(self.d, qc0, qcols, sb_m,
                                    f"mk{self.d}_{qi}")
                if self.leaf:
                    if apply_mask:
                        w = sb_s.tile([128, QUAD * CHUNK], BF16, tag="sq",
                                      name=f"w{self.d}_{qi}")
                        nc.vector.tensor_tensor(w[:, :qcols], z_q,
                                                c_q[:, :qcols], AL.mult)
                        nc.vector.tensor_tensor(hsl, w[:, :qcols],
                                                m_t[:, :qcols], AL.mult)
                    else:
                        nc.vector.tensor_tensor(hsl, z_q, c_q[:, :qcols],
                                                AL.mult)
                else:
                    n0, n1 = qc0 // BLOC, qc1 // BLOC
                    s = sb_s.tile([128, QUAD * CHUNK], BF16, tag="sq",
                                  name=f"s{self.d}_{qi}")
                    nc.vector.tensor_tensor(
                        s[:, :qcols].rearrange("p (n b) -> p n b", b=BLOC),
                        self.hv[:, n0:n1, 0, :], self.hv[:, n0:n1, 1, :],
                        AL.add)
                    if qcols <= CHUNK:
                        # latency-lean 3-op combine for the small-level tail
                        q_q = uq
                        nc.vector.scalar_tensor_tensor(
                            q_q[:, :qcols], s[:, :qcols], -0.5,
                            c_q[:, :qcols], AL.mult, AL.add)
                        w_q = tq
                        nc.vector.tensor_tensor(w_q[:, :qcols], z_q,
                                                q_q[:, :qcols], AL.mult)
                        hdst = hsl
                        if apply_mask:
                            hdst = sb_c.tile([128, QUAD * CHUNK], BF16,
                                             tag="hw", name=f"hw{self.d}_{qi}")
                        tgt = hdst[:, :qcols] if apply_mask else hsl
                        nc.vector.scalar_tensor_tensor(
                            tgt, s[:, :qcols], 0.5, w_q[:, :qcols],
                            AL.mult, AL.add)
                        if apply_mask:
                            nc.vector.tensor_tensor(hsl, hdst[:, :qcols],
                                                    m_t[:, :qcols], AL.mult)
                    else:
                        m = sb_m.tile([128, QUAD * CHUNK], BF16, tag="mq",
                                      name=f"m{self.d}_{qi}")
                        nc.vector.tensor_scalar(m[:, :qcols], s[:, :qcols],
                                                0.5, None, AL.mult)
                        # d = c - m (into uq, consumed by tanh already)
                        d_q = uq
                        nc.vector.tensor_tensor(d_q[:, :qcols], c_q[:, :qcols],
                                                m[:, :qcols], AL.subtract)
                        # e = z' * d (into tq)
                        e_q = tq
                        nc.vector.tensor_tensor(e_q[:, :qcols], z_q,
                                                d_q[:, :qcols], AL.mult)
                        if apply_mask:
                            hw = sb_c.tile([128, QUAD * CHUNK], BF16, tag="hw",
                                           name=f"hw{self.d}_{qi}")
                            nc.vector.tensor_tensor(hw[:, :qcols], m[:, :qcols],
                                                    e_q[:, :qcols], AL.add)
                            nc.vector.tensor_tensor(hsl, hw[:, :qcols],
                                                    m_t[:, :qcols], AL.mult)
                        else:
                            nc.vector.tensor_tensor(hsl, m[:, :qcols],
                                                    e_q[:, :qcols], AL.add)
                self.q_rz[qi] = self.q_t[qi] = self.q_u[qi] = None

        ctxs = {lv["d"]: LevelCtx(lv) for lv in LEVELS}
        next_pair = {d: 0 for d in ctxs}
        flushed_quads = {d: 0 for d in ctxs}
        pendingQ = []

        def ready(d):
            p = next_pair[d]
            if p >= ctxs[d].npairs:
                return False
            if d == 10:
                return True
            if (ctxs[d].nchunks > 4 and d + 2 <= 10
                    and flushed_quads[d + 2] < ctxs[d + 2].nquads):
                return False
            fin = flushed_quads[d + 1] == ctxs[d + 1].nquads
            nq = ctxs[d + 1].nquads
            slack = 2 if nq > 3 else (1 if nq > 1 else 0)
            last = min(2 * p + 1, ctxs[d].nchunks - 1)
            return fin or 2 * (last + 1) <= QUAD * (flushed_quads[d + 1] - slack)

        emit_count = 0
        while True:
            cand = [d for d in range(0, 11) if ready(d)]
            if not cand:
                if pendingQ:
                    for dd, qq, cc in pendingQ:
                        ctxs[dd].emit_quad(qq, cc)
                        flushed_quads[dd] += 1
                    pendingQ = []
                    continue
                break
            d = cand[0]
            # spread leaf pairs as PE filler between internal pairs
            import os as _os
            if (_os.environ.get("KV_LEAFSPREAD", "0") == "1"
                    and 10 in cand and d != 10 and emit_count % 3 == 2):
                d = 10
            emit_count += 1
            done_quad = ctxs[d].emit_pair(next_pair[d])
            next_pair[d] += 1
            # flush quads one emission late so the bulky tanh+combine ops
            # queue behind the next pair's bank-releasing sigmoids
            newQ = []
            if done_quad is not None:
                newQ.append((d, done_quad[0], done_quad[1]))
            for dd, qq, cc in pendingQ:
                ctxs[dd].emit_quad(qq, cc)
                flushed_quads[dd] += 1
            pendingQ = newQ

        # ---- head: mu / logvar from root h ----
        root = hbuf(0)
        ps = ps_hn.tile([128, CHUNK], F32, tag="hn")
        nc.tensor.matmul(ps[:, 0:BLOC], whd_t[:, 0:H], root, start=True, stop=True)
        nc.tensor.matmul(ps[:, BLOC:2 * BLOC], whd_t[:, H:2 * H], root,
                         start=True, stop=True)
        head_sb = singles.tile([128, 2 * BLOC], F32, tag="head")
        nc.scalar.activation(head_sb[:, 0:BLOC], ps[:, 0:BLOC], AF.Identity,
                             bias=mu_b)
        nc.scalar.activation(head_sb[:, BLOC:2 * BLOC], ps[:, BLOC:2 * BLOC],
                             AF.Identity, bias=lv_b)
        nc.sync.dma_start(out=out[0], in_=head_sb[:, 0:BLOC])
        nc.sync.dma_start(out=out[1], in_=head_sb[:, BLOC:2 * BLOC])


# ------------------------- host side -------------------------

def _pack_x(targets, core):
    b0 = core * BLOC
    xp = np.zeros((128, XTOT), NPBF16)
    for lv in LEVELS:
        d, n, L, bs, xoff = lv["d"], lv["n"], lv["L"], lv["bs"], lv["xoff"]
        s = n - 1
        xt = np.ascontiguousarray(
            targets[s:s + n, b0:b0 + BLOC, :].transpose(2, 0, 1).reshape(V, L))
        for bI in range(lv["nblk"]):
            s0, s1 = bI * bs, min((bI + 1) * bs, L)
            xp[64 * bI:64 * bI + V, xoff:xoff + (s1 - s0)] = xt[:, s0:s1].astype(NPBF16)
            xp[64 * bI + V, xoff:xoff + (s1 - s0)] = 1.0
    return xp


def _pack_weights(inp):
    wx = np.zeros((128, 6 * H), np.float32)
    for bI, base in enumerate((0, 64)):
        o = 3 * bI * H
        wx[base:base + V, o:o + H] = inp["wir_w"].T
        wx[base + V, o:o + H] = inp["wir_b"] + inp["whr_b"]
        wx[base:base + V, o + H:o + 2 * H] = -inp["wiz_w"].T
        wx[base + V, o + H:o + 2 * H] = -(inp["wiz_b"] + inp["whz_b"])
        wx[base:base + V, o + 2 * H:o + 3 * H] = inp["win_w"].T
        wx[base + V, o + 2 * H:o + 3 * H] = inp["win_b"]

    wh = np.zeros((128, 6 * H), np.float32)
    wh[:, 0:H] = inp["whr_w"][:, :H].T
    wh[:, H:2 * H] = inp["whr_w"][:, H:].T
    wh[:, 2 * H:3 * H] = -inp["whz_w"][:, :H].T
    wh[:, 3 * H:4 * H] = -inp["whz_w"][:, H:].T
    wh[:, 4 * H:5 * H] = inp["whn_w"][:, :H].T
    wh[:, 5 * H:6 * H] = inp["whn_w"][:, H:].T

    whd = np.zeros((128, 2 * H), np.float32)
    whd[:, 0:H] = inp["mu_w"].T
    whd[:, H:2 * H] = inp["lv_w"].T

    bias = np.zeros((128, 3), np.float32)
    bias[:, 0] = inp["whn_b"]
    bias[:, 1] = inp["mu_b"]
    bias[:, 2] = inp["lv_b"]

    return {"wx": wx.astype(NPBF16), "wh": wh.astype(NPBF16),
            "whd": whd.astype(NPBF16), "bias": bias}


_NC_CACHE = {}
TRACE = False
LAST_RES = None


def kernel(**inputs):
    global LAST_RES
    from concourse.bass_utils import run_bass_kernel_spmd

    targets = np.asarray(inputs["targets"], np.float32)
    masks = np.asarray(inputs["masks"], np.float32)
    apply_mask = not bool(np.all(masks == 1.0))

    if apply_mask not in _NC_CACHE:
        _NC_CACHE[apply_mask] = build_nc(apply_mask)
    nc = _NC_CACHE[apply_mask]

    weights = _pack_weights({k: np.asarray(v, np.float32)
                             for k, v in inputs.items()
                             if k not in ("targets", "masks")})
    in_maps = []
    for core in range(NCORES):
        m = {"xp": _pack_x(targets, core)}
        m.update(weights)
        if apply_mask:
            b0 = core * BLOC
            m["mrow"] = np.ascontiguousarray(
                masks[:, b0:b0 + BLOC]).reshape(1, N_NODES * BLOC)
        in_maps.append(m)

    res = run_bass_kernel_spmd(nc, in_maps, list(range(NCORES)), trace=TRACE)
    LAST_RES = res
    mu = np.empty((B, H), np.float32)
    lvr = np.empty((B, H), np.float32)
    for core in range(NCORES):
        o = res.results[core]["out"]
        mu[core * BLOC:(core + 1) * BLOC] = o[0].T
        lvr[core * BLOC:(core + 1) * BLOC] = o[1].T
    return mu, lvr


if __name__ == "__main__":
    build_nc(False)
    print("built ok; XTOT =", XTOT)


# revision 6
# speedup vs baseline: 1.3133x; 1.3133x over previous
"""Trainium2 Bass kernel for nn_Encoder_40535901340423 (binary-tree GRU encoder).

v2: same sharding/host packing as baseline (batch/8, feature-major on chip),
rewritten device schedule:
  - PE: weight-major over chunk PAIRS (one LDWEIGHTS serves 2 matmuls),
    512-col chunks, 4 PSUM banks per chunk, 2 chunks in flight.
  - ACT: one sigmoid per chunk over [r|z'] (1024 cols, PSUM src),
    one tanh per quad (<=2048 cols, SBUF).
  - DVE only for elementwise (no gpsimd compute - Pool ops poison DVE
    SBUF ports): all-bf16-SBUF tensor_tensor (2x mode) / tensor_scalar
    (4x) where possible; PSUM-source ops kept to t/u per chunk.
  - combine: s=h1+h2, m=0.5s, d=c-m, e=z'*d, h=m+e (all TT/TS).
"""

import math
import sys

import numpy as np
import ml_dtypes

if "/opt/trn_rl_repo" not in sys.path:
    sys.path.insert(0, "/opt/trn_rl_repo")

import concourse.bass as bass
from concourse import bacc
import concourse.mybir as mybir
import concourse.tile as tile

N_NODES, B, V, H = 2047, 128, 32, 128
NCORES = 8
BLOC = B // NCORES  # 16
CHUNK = 512
QUAD = 4  # chunks per combine quad
F32 = mybir.dt.float32
BF16 = mybir.dt.bfloat16
NPBF16 = ml_dtypes.bfloat16
AL = mybir.AluOpType
AF = mybir.ActivationFunctionType


def _level_meta():
    meta = []
    off = 0
    for d in range(10, -1, -1):
        n = 1 << d
        L = n * BLOC
        if L >= 2 * CHUNK:
            bs = int(math.ceil(L / 2 / CHUNK)) * CHUNK
        else:
            bs = L
        nblk = int(math.ceil(L / bs))
        meta.append(dict(d=d, n=n, L=L, bs=bs, nblk=nblk, xoff=off))
        off += bs
    return meta, off


LEVELS, XTOT = _level_meta()


def build_nc(apply_mask: bool):
    nc = bacc.Bacc()
    xp = nc.declare_dram_parameter("xp", [128, XTOT], BF16, isOutput=False)
    wx = nc.declare_dram_parameter("wx", [128, 6 * H], BF16, isOutput=False)
    wh = nc.declare_dram_parameter("wh", [128, 6 * H], BF16, isOutput=False)
    whd = nc.declare_dram_parameter("whd", [128, 2 * H], BF16, isOutput=False)
    bia = nc.declare_dram_parameter("bias", [128, 3], F32, isOutput=False)
    mrow = None
    if apply_mask:
        mrow = nc.declare_dram_parameter(
            "mrow", [1, N_NODES * BLOC], F32, isOutput=False)
    out = nc.declare_dram_parameter("out", [2, 128, BLOC], F32, isOutput=True)

    with tile.TileContext(nc) as tc:
        _emit(tc, nc, xp, wx, wh, whd, bia, mrow, out, apply_mask)
    if not nc.is_finalized():
        nc.finalize()
    return nc


def _emit(tc, nc, xp, wx, wh, whd, bia, mrow, out, apply_mask):
    import contextlib

    with contextlib.ExitStack() as ctx:
        singles = ctx.enter_context(tc.tile_pool(name="singles", bufs=1))
        hbufs = ctx.enter_context(tc.tile_pool(name="hbufs", bufs=1))
        ps_rz = ctx.enter_context(tc.tile_pool(name="ps_rz", bufs=2, space="PSUM"))
        ps_hn = ctx.enter_context(tc.tile_pool(name="ps_hn", bufs=2, space="PSUM"))
        ps_xn = ctx.enter_context(tc.tile_pool(name="ps_xn", bufs=2, space="PSUM"))
        sb_rz = ctx.enter_context(tc.tile_pool(name="sb_rz", bufs=2))
        sb_t = ctx.enter_context(tc.tile_pool(name="sb_t", bufs=2))
        sb_u = ctx.enter_context(tc.tile_pool(name="sb_u", bufs=2))
        sb_c = ctx.enter_context(tc.tile_pool(name="sb_c", bufs=2))
        sb_s = ctx.enter_context(tc.tile_pool(name="sb_s", bufs=2))
        sb_m = ctx.enter_context(tc.tile_pool(name="sb_m", bufs=2))

        # --- resident tensors ---
        wx_t = singles.tile([128, 6 * H], BF16, tag="wx")
        nc.sync.dma_start(out=wx_t[:, :], in_=wx[:, :])
        wh_t = singles.tile([128, 6 * H], BF16, tag="wh")
        nc.sync.dma_start(out=wh_t[:, :], in_=wh[:, :])
        whd_t = singles.tile([128, 2 * H], BF16, tag="whd")
        nc.sync.dma_start(out=whd_t[:, :], in_=whd[:, :])
        bia_t = singles.tile([128, 3], F32, tag="bias")
        nc.sync.dma_start(out=bia_t[:, :], in_=bia[:, :])
        x_res = singles.tile([128, XTOT], BF16, tag="x_res")
        for lv in LEVELS:
            half = lv["bs"] // 8 if lv["d"] == 10 else lv["bs"]
            for p0 in range(lv["xoff"], lv["xoff"] + lv["bs"], max(half, 16)):
                p1 = min(p0 + max(half, 16), lv["xoff"] + lv["bs"])
                nc.sync.dma_start(out=x_res[:, p0:p1], in_=xp[:, p0:p1])
        whn_b = bia_t[:, 0:1]
        mu_b = bia_t[:, 1:2]
        lv_b = bia_t[:, 2:3]

        ping = hbufs.tile([128, 16384], BF16, tag="ping")
        pong = hbufs.tile([128, 8192], BF16, tag="pong")

        def hbuf(d):
            L = (1 << d) * BLOC
            return (ping if (10 - d) % 2 == 0 else pong)[:, :L]

        WXg = [[wx_t[:, (3 * bI + g) * H:(3 * bI + g + 1) * H]
                for g in range(3)] for bI in range(2)]
        Ug = [wh_t[:, (2 * g) * H:(2 * g + 1) * H] for g in range(3)]
        Vg = [wh_t[:, (2 * g + 1) * H:(2 * g + 2) * H] for g in range(3)]

        def mask_tile(d, c0, sz, pool, nm):
            n = 1 << d
            start = (n - 1) * BLOC
            m_t = pool.tile([128, QUAD * CHUNK], F32, tag="m", name=nm)
            src = mrow[0:1, start + c0: start + c0 + sz]
            bsrc = bass.AP(tensor=src.tensor, offset=src.offset,
                           ap=[[0, 128]] + list(src.ap[1:]))
            nc.sync.dma_start(out=m_t[:, :sz], in_=bsrc)
            return m_t

        class LevelCtx:
            def __init__(self, lv):
                self.lv = lv
                self.d, self.L = lv["d"], lv["L"]
                self.bs, self.xoff = lv["bs"], lv["xoff"]
                self.leaf = self.d == 10
                self.h_out = hbuf(self.d)
                self.hv = None
                if not self.leaf:
                    self.hv = hbuf(self.d + 1).rearrange(
                        "p (n two b) -> p n two b", two=2, b=BLOC)
                self.nchunks = int(math.ceil(self.L / CHUNK))
                self.chunks = [(i * CHUNK, min((i + 1) * CHUNK, self.L))
                               for i in range(self.nchunks)]
                if 64 <= self.L <= CHUNK:
                    half = -(-self.L // 2 // BLOC) * BLOC
                    self.chunks = [(0, half), (half, self.L)]
                    self.nchunks = 2
                self.npairs = int(math.ceil(self.nchunks / 2))
                self.nquads = int(math.ceil(self.nchunks / QUAD))
                # per-quad state: rz/t/u tiles
                self.q_rz = [None] * self.nquads
                self.q_t = [None] * self.nquads
                self.q_u = [None] * self.nquads

            def xin(self, g, c0, c1):
                """(lhsT, rhs) for the x matmul of gate g over level cols
                [c0,c1) (must lie within one block)."""
                bI = c0 // self.bs
                o = self.xoff + c0 - bI * self.bs
                return (WXg[bI][g], x_res[:, o:o + (c1 - c0)])

            def quad_tiles(self, qi):
                if self.q_rz[qi] is None:
                    self.q_rz[qi] = sb_rz.tile(
                        [128, 2 * QUAD * CHUNK], BF16, tag="rzq",
                        name=f"rzq{self.d}_{qi}")
                    self.q_t[qi] = sb_t.tile(
                        [128, QUAD * CHUNK], BF16, tag="tq",
                        name=f"tq{self.d}_{qi}")
                    self.q_u[qi] = sb_u.tile(
                        [128, QUAD * CHUNK], BF16, tag="uq",
                        name=f"uq{self.d}_{qi}")
                return self.q_rz[qi], self.q_t[qi], self.q_u[qi]

            def emit_pair(self, pi):
                """Emit PE + sigmoid + t/u for chunks 2pi, 2pi+1."""
                cidx = [c for c in (2 * pi, 2 * pi + 1) if c < self.nchunks]
                chs = [self.chunks[c] for c in cidx]
                leaf = self.leaf
                rzs = []
                hns = []
                xns = []
                kids = []
                for (c0, c1) in chs:
                    sz = c1 - c0
                    rzs.append(ps_rz.tile([128, 2 * CHUNK], F32, tag="rz",
                                          name=f"rz{self.d}_{c0}"))
                    xns.append(ps_xn.tile([128, CHUNK], F32, tag="xn",
                                          name=f"xn{self.d}_{c0}"))
                    if not leaf:
                        hns.append(ps_hn.tile([128, CHUNK], F32, tag="hn",
                                              name=f"hn{self.d}_{c0}"))
                        n0, n1 = c0 // BLOC, c1 // BLOC
                        kids.append((self.hv[:, n0:n1, 0, :],
                                     self.hv[:, n0:n1, 1, :]))
                # r then z' gates: x (start) -> U -> V (stop), weight-major
                for g in range(2):
                    for k, (c0, c1) in enumerate(chs):
                        sz = c1 - c0
                        lhsT, rhs = self.xin(g, c0, c1)
                        nc.tensor.matmul(rzs[k][:, g * sz:(g + 1) * sz],
                                         lhsT, rhs, start=True, stop=leaf)
                    if not leaf:
                        for k, (c0, c1) in enumerate(chs):
                            sz = c1 - c0
                            nc.tensor.matmul(rzs[k][:, g * sz:(g + 1) * sz],
                                             Ug[g], kids[k][0],
                                             start=False, stop=False)
                        for k, (c0, c1) in enumerate(chs):
                            sz = c1 - c0
                            nc.tensor.matmul(rzs[k][:, g * sz:(g + 1) * sz],
                                             Vg[g], kids[k][1],
                                             start=False, stop=True)
                # sigmoid per chunk: [r|z'] -> rz quad tile (r half / z half)
                for k, c in enumerate(cidx):
                    c0, c1 = self.chunks[c]
                    sz = c1 - c0
                    qi, qslot = c // QUAD, c % QUAD
                    rzq, _, _ = self.quad_tiles(qi)
                    qoff = c0 - self.chunks[qi * QUAD][0]
                    rz_out = rzq.rearrange("p (b c) -> p b c", b=2)[
                        :, :, qoff:qoff + sz]
                    rz_in = rzs[k][:, :2 * sz].rearrange(
                        "p (b c) -> p b c", b=2)
                    nc.scalar.activation(rz_out, rz_in, AF.Sigmoid)
                # xn gate
                for k, (c0, c1) in enumerate(chs):
                    sz = c1 - c0
                    lhsT, rhs = self.xin(2, c0, c1)
                    nc.tensor.matmul(xns[k][:, :sz], lhsT, rhs,
                                     start=True, stop=True)
                if not leaf:
                    for k, (c0, c1) in enumerate(chs):
                        sz = c1 - c0
                        nc.tensor.matmul(hns[k][:, :sz], Ug[2], kids[k][0],
                                         start=True, stop=False)
                    for k, (c0, c1) in enumerate(chs):
                        sz = c1 - c0
                        nc.tensor.matmul(hns[k][:, :sz], Vg[2], kids[k][1],
                                         start=False, stop=True)
                # t / u per chunk on DVE
                for k, c in enumerate(cidx):
                    c0, c1 = self.chunks[c]
                    sz = c1 - c0
                    qi = c // QUAD
                    rzq, tq, uq = self.quad_tiles(qi)
                    qoff = c0 - self.chunks[qi * QUAD][0]
                    r_sl = rzq[:, qoff:qoff + sz]
                    if leaf:
                        # u = r*whn_b + xn
                        nc.vector.scalar_tensor_tensor(
                            uq[:, qoff:qoff + sz], r_sl, whn_b,
                            xns[k][:, :sz], AL.mult, AL.add)
                    else:
                        # t = (hn + whn_b) * r ; u = t + xn
                        nc.vector.scalar_tensor_tensor(
                            tq[:, qoff:qoff + sz], hns[k][:, :sz], whn_b,
                            r_sl, AL.add, AL.mult)
                        nc.vector.tensor_tensor(
                            uq[:, qoff:qoff + sz], tq[:, qoff:qoff + sz],
                            xns[k][:, :sz], AL.add)
                last = cidx[-1]
                if last % QUAD == QUAD - 1 or last == self.nchunks - 1:
                    return (last // QUAD, last)
                return None

            def emit_quad(self, qi, ci_last):
                qc0 = self.chunks[qi * QUAD][0]
                qc1 = self.chunks[ci_last][1]
                qcols = qc1 - qc0
                rzq, tq, uq = self.quad_tiles(qi)
                z_q = rzq[:, QUAD * CHUNK:QUAD * CHUNK + qcols]
                c_q = sb_c.tile([128, QUAD * CHUNK], BF16, tag="cq",
                                name=f"cq{self.d}_{qi}")
                nc.scalar.activation(c_q[:, :qcols], uq[:, :qcols], AF.Tanh)
                hsl = self.h_out[:, qc0:qc1]
                m_t = None
                if apply_mask:
                    m_t = mask_tile(self.d, qc0, qcols, sb_m,
                                    f"mk{self.d}_{qi}")
                if self.leaf:
                    if apply_mask:
                        w = sb_s.tile([128, QUAD * CHUNK], BF16, tag="sq",
                                      name=f"w{self.d}_{qi}")
                        nc.vector.tensor_tensor(w[:, :qcols], z_q,
                                                c_q[:, :qcols], AL.mult)
                        nc.vector.tensor_tensor(hsl, w[:, :qcols],
                                                m_t[:, :qcols], AL.mult)
                    else:
                        nc.vector.tensor_tensor(hsl, z_q, c_q[:, :qcols],
                                                AL.mult)
                else:
                    n0, n1 = qc0 // BLOC, qc1 // BLOC
                    s = sb_s.tile([128, QUAD * CHUNK], BF16, tag="sq",
                                  name=f"s{self.d}_{qi}")
                    nc.vector.tensor_tensor(
                        s[:, :qcols].rearrange("p (n b) -> p n b", b=BLOC),
                        self.hv[:, n0:n1, 0, :], self.hv[:, n0:n1, 1, :],
                        AL.add)
                    if qcols <= CHUNK:
                        # latency-lean 3-op combine for the small-level tail
                        q_q = uq
                        nc.vector.scalar_tensor_tensor(
                            q_q[:, :qcols], s[:, :qcols], -0.5,
                            c_q[:, :qcols], AL.mult, AL.add)
                        w_q = tq
                        nc.vector.tensor_tensor(w_q[:, :qcols], z_q,
                                                q_q[:, :qcols], AL.mult)
                        hdst = hsl
                        if apply_mask:
                            hdst = sb_c.tile([128, QUAD * CHUNK], BF16,
                                             tag="hw", name=f"hw{self.d}_{qi}")
                        tgt = hdst[:, :qcols] if apply_mask else hsl
                        nc.vector.scalar_tensor_tensor(
                            tgt, s[:, :qcols], 0.5, w_q[:, :qcols],
                            AL.mult, AL.add)
                        if apply_mask:
                            nc.vector.tensor_tensor(hsl, hdst[:, :qcols],
                                                    m_t[:, :qcols], AL.mult)
                    else:
                        m = sb_m.tile([128, QUAD * CHUNK], BF16, tag="mq",
                                      name=f"m{self.d}_{qi}")
                        nc.vector.tensor_scalar(m[:, :qcols], s[:, :qcols],
                                                0.5, None, AL.mult)
                        # d = c - m (into uq, consumed by tanh already)
                        d_q = uq
                        nc.vector.tensor_tensor(d_q[:, :qcols], c_q[:, :qcols],
                                                m[:, :qcols], AL.subtract)
                        # e = z' * d (into tq)
                        e_q = tq
                        nc.vector.tensor_tensor(e_q[:, :qcols], z_q,
                                                d_q[:, :qcols], AL.mult)
                        if apply_mask:
                            hw = sb_c.tile([128, QUAD * CHUNK], BF16, tag="hw",
                                           name=f"hw{self.d}_{qi}")
                            nc.vector.tensor_tensor(hw[:, :qcols], m[:, :qcols],
                                                    e_q[:, :qcols], AL.add)
                            nc.vector.tensor_tensor(hsl, hw[:, :qcols],
                                                    m_t[:, :qcols], AL.mult)
                        else:
                            nc.vector.tensor_tensor(hsl, m[:, :qcols],
                                                    e_q[:, :qcols], AL.add)
                self.q_rz[qi] = self.q_t[qi] = self.q_u[qi] = None

        ctxs = {lv["d"]: LevelCtx(lv) for lv in LEVELS}
        next_pair = {d: 0 for d in ctxs}
        flushed_quads = {d: 0 for d in ctxs}
        pendingQ = []

        def ready(d):
            p = next_pair[d]
            if p >= ctxs[d].npairs:
                return False
            if d == 10:
                return True
            if (ctxs[d].nchunks > 4 and d + 2 <= 10
                    and flushed_quads[d + 2] < ctxs[d + 2].nquads):
                return False
            fin = flushed_quads[d + 1] == ctxs[d + 1].nquads
            nq = ctxs[d + 1].nquads
            slack = 2 if nq > 3 else (1 if nq > 1 else 0)
            last = min(2 * p + 1, ctxs[d].nchunks - 1)
            return fin or 2 * (last + 1) <= QUAD * (flushed_quads[d + 1] - slack)

        emit_count = 0
        while True:
            cand = [d for d in range(0, 11) if ready(d)]
            if not cand:
                if pendingQ:
                    for dd, qq, cc in pendingQ:
                        ctxs[dd].emit_quad(qq, cc)
                        flushed_quads[dd] += 1
                    pendingQ = []
                    continue
                break
            d = cand[0]
            emit_count += 1
            done_quad = ctxs[d].emit_pair(next_pair[d])
            next_pair[d] += 1
            if done_quad is not None:
                ctxs[d].emit_quad(done_quad[0], done_quad[1])
                flushed_quads[d] += 1

        # ---- head: mu / logvar from root h ----
        root = hbuf(0)
        ps = ps_hn.tile([128, CHUNK], F32, tag="hn")
        nc.tensor.matmul(ps[:, 0:BLOC], whd_t[:, 0:H], root, start=True, stop=True)
        nc.tensor.matmul(ps[:, BLOC:2 * BLOC], whd_t[:, H:2 * H], root,
                         start=True, stop=True)
        head_sb = singles.tile([128, 2 * BLOC], F32, tag="head")
        nc.scalar.activation(head_sb[:, 0:BLOC], ps[:, 0:BLOC], AF.Identity,
                             bias=mu_b)
        nc.scalar.activation(head_sb[:, BLOC:2 * BLOC], ps[:, BLOC:2 * BLOC],
                             AF.Identity, bias=lv_b)
        nc.sync.dma_start(out=out[0], in_=head_sb[:, 0:BLOC])
        nc.sync.dma_start(out=out[1], in_=head_sb[:, BLOC:2 * BLOC])


# ------------------------- host side -------------------------

def _pack_x(targets, core):
    b0 = core * BLOC
    xp = np.zeros((128, XTOT), NPBF16)
    for lv in LEVELS:
        d, n, L, bs, xoff = lv["d"], lv["n"], lv["L"], lv["bs"], lv["xoff"]
        s = n - 1
        xt = np.ascontiguousarray(
            targets[s:s + n, b0:b0 + BLOC, :].transpose(2, 0, 1).reshape(V, L))
        for bI in range(lv["nblk"]):
            s0, s1 = bI * bs, min((bI + 1) * bs, L)
            xp[64 * bI:64 * bI + V, xoff:xoff + (s1 - s0)] = xt[:, s0:s1].astype(NPBF16)
            xp[64 * bI + V, xoff:xoff + (s1 - s0)] = 1.0
    return xp


def _pack_weights(inp):
    wx = np.zeros((128, 6 * H), np.float32)
    for bI, base in enumerate((0, 64)):
        o = 3 * bI * H
        wx[base:base + V, o:o + H] = inp["wir_w"].T
        wx[base + V, o:o + H] = inp["wir_b"] + inp["whr_b"]
        wx[base:base + V, o + H:o + 2 * H] = -inp["wiz_w"].T
        wx[base + V, o + H:o + 2 * H] = -(inp["wiz_b"] + inp["whz_b"])
        wx[base:base + V, o + 2 * H:o + 3 * H] = inp["win_w"].T
        wx[base + V, o + 2 * H:o + 3 * H] = inp["win_b"]

    wh = np.zeros((128, 6 * H), np.float32)
    wh[:, 0:H] = inp["whr_w"][:, :H].T
    wh[:, H:2 * H] = inp["whr_w"][:, H:].T
    wh[:, 2 * H:3 * H] = -inp["whz_w"][:, :H].T
    wh[:, 3 * H:4 * H] = -inp["whz_w"][:, H:].T
    wh[:, 4 * H:5 * H] = inp["whn_w"][:, :H].T
    wh[:, 5 * H:6 * H] = inp["whn_w"][:, H:].T

    whd = np.zeros((128, 2 * H), np.float32)
    whd[:, 0:H] = inp["mu_w"].T
    whd[:, H:2 * H] = inp["lv_w"].T

    bias = np.zeros((128, 3), np.float32)
    bias[:, 0] = inp["whn_b"]
    bias[:, 1] = inp["mu_b"]
    bias[:, 2] = inp["lv_b"]

    return {"wx": wx.astype(NPBF16), "wh": wh.astype(NPBF16),
            "whd": whd.astype(NPBF16), "bias": bias}


_NC_CACHE = {}
TRACE = False
LAST_RES = None


def kernel(**inputs):
    global LAST_RES
    from concourse.bass_utils import run_bass_kernel_spmd

    targets = np.asarray(inputs["targets"], np.float32)
    masks = np.asarray(inputs["masks"], np.float32)
    apply_mask = not bool(np.all(masks == 1.0))

    if apply_mask not in _NC_CACHE:
        _NC_CACHE[apply_mask] = build_nc(apply_mask)
    nc = _NC_CACHE[apply_mask]

    weights = _pack_weights({k: np.asarray(v, np.float32)
                             for k, v in inputs.items()
                             if k not in ("targets", "masks")})
    in_maps = []
    for core in range(NCORES):
        m = {"xp": _pack_x(targets, core)}
        m.update(weights)
        if apply_mask:
            b0 = core * BLOC
            m["mrow"] = np.ascontiguousarray(
                masks[:, b0:b0 + BLOC]).reshape(1, N_NODES * BLOC)
        in_maps.append(m)

    res = run_bass_kernel_spmd(nc, in_maps, list(range(NCORES)), trace=TRACE)
    LAST_RES = res
    mu = np.empty((B, H), np.float32)
    lvr = np.empty((B, H), np.float32)
    for core in range(NCORES):
        o = res.results[core]["out"]
        mu[core * BLOC:(core + 1) * BLOC] = o[0].T
        lvr[core * BLOC:(core + 1) * BLOC] = o[1].T
    return mu, lvr


if __name__ == "__main__":
    build_nc(False)
    print("built ok; XTOT =", XTOT)
